# revision 5
# baseline (speedup 1.0000x reference)
"""Causal multi-head attention on 8 Trainium2 NeuronCores (v2, fp8 DoubleRow).

Full module: x:(2,2048,1024) f32, 16 heads, head_dim 64, causal softmax
(scaled by 1/sqrt(1024)), out = attn(x) @ Wo + bo.

Sharding: core c handles batch b = c // 4 and head group g = c % 4
(4 heads, i.e. 256 columns of Wq/Wk/Wv and 256 rows of Wo). Every core
runs the same program (SPMD); the host sums the 4 per-group partial
output projections per batch (f16 partials) and adds the bias.

v2 design vs baseline:
  * host supplies x pre-transposed: xt bf16 [128,8,N] (V proj) and
    x8 fp8e4m3 [128,4,2,N] (Q/K proj, DoubleRow k-tile-pair layout) --
    no PE transposes on device.
  * Q/K projections run fp8 DoubleRow (2 f-chunks per pass, 0.5 cyc/row):
    host scales Wq/Wk by 32 (fp8-friendly range) and permutes their
    columns so the projection psum comes out partition-ordered as
    head*32+d_local with d-halves split across the two dh matmuls.
  * S = K^T Q runs fp8 DoubleRow per head: qt8/kt8 [32h:32h+32, 2, N]
    hold head h's d-halves as the 2 k-tiles; explicit tile_position
    (32h, 0) row-tiles the PE array. exp scale absorbs the 32*32 weight
    scaling (exp(S_psum / 32768)).
  * exp activations trimmed to the causal-valid column range; the
    causal mask (gpsimd affine_select, [128,128] diagonal block only)
    is issued right after exp and consumed by PV one chunk later, so
    Pool latency stays off the critical path. PV of diagonal chunks is
    split into a clean part (no mask dep) and the masked 128-col block.
  * proj / out-proj / epilogue-normalize steps live in a global work
    queue; one step is popped between S(kc) and PV(kc-1) of every
    attention chunk to fill PE slack while ScalarE runs exp.
  * V path / PV / out-proj stay bf16 (fp8 there breaks the 2e-2 gate).
  * out is written as f16 partials (halves output DMA); host upcasts.
"""

import os

import numpy as np

N = 2048        # tokens per batch
D = 1024        # model dim
HG = 4          # heads per core
HD = 64         # head dim
DG = HG * HD    # 256, feature columns per core
NCORES = 8
NT = N // 128   # 16 token chunks
NF = D // 128   # 8 feature chunks
NQ = N // 512   # 4 query blocks
# host scales Wq,Wk by 32 -> S_psum = 1024 * S; module scale 1/sqrt(1024)
EXP_SCALE = 1.0 / (32.0 * 1024.0)

PT_BUFS = int(os.environ.get("PT_BUFS", "10"))

_CACHE = {}


def _build_nc(repeat=1):
    from contextlib import ExitStack

    import concourse.mybir as mybir
    import concourse.tile as tile
    from concourse import bacc

    FP32 = mybir.dt.float32
    F32R = mybir.dt.float32r
    FP16 = mybir.dt.float16
    BF16 = mybir.dt.bfloat16
    F8 = mybir.dt.float8e4
    EXP = mybir.ActivationFunctionType.Exp
    COPY = mybir.ActivationFunctionType.Copy
    DR = mybir.MatmulPerfMode.DoubleRow

    nc = bacc.Bacc("TRN2", target_bir_lowering=False, debug=False)

    x8_d = nc.dram_tensor("x8", [128, NF // 2, 2, N], F8, kind="ExternalInput").ap()
    xt_d = nc.dram_tensor("xt", [128, NF, N], BF16, kind="ExternalInput").ap()
    wq_d = nc.dram_tensor("wq8", [128, NF // 2, 2, DG], F8, kind="ExternalInput").ap()
    wk_d = nc.dram_tensor("wk8", [128, NF // 2, 2, DG], F8, kind="ExternalInput").ap()
    wv_d = nc.dram_tensor("wv", [128, NF, DG], BF16, kind="ExternalInput").ap()
    wo_d = nc.dram_tensor("wo", [128, 2, D], BF16, kind="ExternalInput").ap()
    out_d = nc.dram_tensor("out", [N, D], FP16, kind="ExternalOutput").ap()

    with tile.TileContext(nc) as tc, ExitStack() as ctx:
        persist = ctx.enter_context(tc.tile_pool(name="persist", bufs=1))
        ptpool = ctx.enter_context(tc.tile_pool(name="ptpool", bufs=PT_BUFS))
        stpool = ctx.enter_context(tc.tile_pool(name="stpool", bufs=6))
        smpool = ctx.enter_context(tc.tile_pool(name="smpool", bufs=4))
        opool = ctx.enter_context(tc.tile_pool(name="opool", bufs=3))
        # PSUM budget (8 banks): proj/outproj/bc "ps" [128,512]x2 = 2 banks;
        # attention S "ps_s" [128,1024]x2 = 4 banks; pv accumulators 2 banks.
        mmpsum = ctx.enter_context(tc.tile_pool(name="mmpsum", bufs=2, space="PSUM"))
        spsum = mmpsum
        pvpsum = ctx.enter_context(tc.tile_pool(name="pvpsum", bufs=1, space="PSUM"))

        # ---- persistent tensors ----
        x8 = persist.tile([128, NF // 2, 2, N], F8, name="x8")    # 16 KB/p
        xT = persist.tile([128, NF, N], BF16, name="xT")          # 32 KB/p
        # qt8/kt8/vt are double-buffered by body parity: the NEXT body's
        # projections may then run during THIS body's last attention block
        # (whose S/PV still read the old buffers), so with in-NEFF
        # repetition the exp stream never waits at the body boundary.
        qt8s = [persist.tile([128, 2, N], F8, name=f"qt8_{i}")
                for i in range(2)]                                # 4 KB/p x2
        kt8s = [persist.tile([128, 2, N], F8, name=f"kt8_{i}")
                for i in range(2)]                                # 4 KB/p x2
        vts = [persist.tile([128, NT, HG, HD + 1], BF16, name=f"vt_{i}")
               for i in range(2)]                                 # ~8 KB/p x2
        cur = {"qt8": qt8s[0], "kt8": kt8s[0], "vt": vts[0]}
        ctxT = persist.tile([128, 2, N], BF16, name="ctxT")       # 8 KB/p
        wq8 = persist.tile([128, NF // 2, 2, DG], F8, name="wq8")  # 2 KB/p
        wk8 = persist.tile([128, NF // 2, 2, DG], F8, name="wk8")
        wv_bf = persist.tile([128, NF, DG], BF16, name="wv_bf")   # 4 KB/p
        wo_bf = persist.tile([128, 2, D], BF16, name="wo_bf")     # 4 KB/p
        ones128 = persist.tile([128, HD], F32R, name="ones128")

        # walrus requires f32r operands produced by a rounding op
        ones_f32 = persist.tile([128, HD], FP32, name="ones_f32")
        nc.gpsimd.memset(ones_f32[:, :], 1.0)
        nc.vector.tensor_copy(ones128[:, :], ones_f32[:, :])
        for _vt in vts:
            nc.gpsimd.memset(_vt[:, :, :, HD], 1.0)  # softmax-sum ones cols

        def emit_weights_vo():
            nc.sync.dma_start(out=wv_bf[:, :, :], in_=wv_d)
            nc.sync.dma_start(out=wo_bf[:, :, :], in_=wo_d)

        def emit_x8_dma(ib, half=None):
            if half is None:
                tsl = slice(512 * ib, 512 * (ib + 1))
            else:
                tsl = slice(512 * ib + 256 * half, 512 * ib + 256 * (half + 1))
            nc.sync.dma_start(out=x8[:, :, :, tsl], in_=x8_d[:, :, :, tsl])

        def emit_xt_dma(ib):
            tsl = slice(512 * ib, 512 * (ib + 1))
            nc.sync.dma_start(out=xT[:, :, tsl], in_=xt_d[:, :, tsl])

        def make_qk_steps(ib, only=None):
            """4 steps: (Q|K) x (dh 0|1). th outer so the first half of
            the x8 slab suffices to start. only='q'|'k' selects half."""
            tsl = slice(512 * ib, 512 * (ib + 1))
            steps = []

            def qk_step(w8, dst, dh):
                def go():
                    ps = mmpsum.tile([128, 512], FP32, name="ps", tag="ps")
                    for th in range(2):
                        for j in range(NF // 2):
                            nc.tensor.matmul(
                                ps[:, 256 * th:256 * (th + 1)],
                                lhsT=w8[:, j, :, 128 * dh:128 * (dh + 1)],
                                rhs=x8[:, j, :,
                                       512 * ib + 256 * th:
                                       512 * ib + 256 * (th + 1)],
                                start=(j == 0), stop=(j == NF // 2 - 1),
                                perf_mode=DR,
                            )
                    nc.vector.tensor_copy(cur[dst][:, dh, tsl], ps[:, :])
                return go

            pairs = {"q": ((wq8, "qt8"),), "k": ((wk8, "kt8"),),
                     None: ((wq8, "qt8"), (wk8, "kt8"))}[only]
            for w8, dst in pairs:
                for dh in range(2):
                    steps.append(qk_step(w8, dst, dh))
            return steps

        def make_v_steps(ib, tccs=None):
            steps = []

            def v_step(tcc):
                def go():
                    ps = mmpsum.tile([128, 512], FP32, name="ps", tag="ps")
                    for fc in range(NF):
                        nc.tensor.matmul(
                            ps[:, 0:DG],
                            lhsT=xT[:, fc, 128 * tcc:128 * (tcc + 1)],
                            rhs=wv_bf[:, fc, :],
                            start=(fc == 0), stop=(fc == NF - 1),
                        )
                    # (gpsimd cannot read PSUM -- must stay on DVE)
                    nc.vector.tensor_copy(
                        cur["vt"][:, tcc, :, 0:HD],
                        ps[:, 0:DG].rearrange("p (h e) -> p h e", h=HG))
                return go

            for tcc in (tccs if tccs is not None
                        else range(4 * ib, 4 * ib + 4)):
                steps.append(v_step(tcc))
            return steps

        def make_outproj_steps(qb):
            """8 steps: (tb, nh); partial over heads, host sums groups."""
            steps = []

            def o_step(tb, nh):
                def go():
                    tsl = slice(128 * tb, 128 * (tb + 1))
                    ps_o = mmpsum.tile([128, 512], FP32, name="ps", tag="ps")
                    for hc in range(2):
                        nc.tensor.matmul(
                            ps_o[:, :],
                            lhsT=ctxT[:, hc, tsl],
                            rhs=wo_bf[:, hc, 512 * nh:512 * (nh + 1)],
                            start=(hc == 0), stop=(hc == 1),
                        )
                    o_sb = opool.tile([128, 512], FP16, name="o_sb")
                    # tail block: exps are done, alternate ACT/DVE eviction
                    # and spread the out DMAs over a second queue
                    if qb == NQ - 1 and nh == 0:
                        nc.scalar.activation(o_sb[:, :], ps_o[:, :], COPY)
                    else:
                        nc.vector.tensor_copy(o_sb[:, :], ps_o[:, :])
                    eng = nc.scalar if qb == NQ - 1 and nh == 1 else nc.sync
                    eng.dma_start(
                        out=out_d[tsl, 512 * nh:512 * (nh + 1)],
                        in_=o_sb[:, :])
                return go

            for tb in range(4 * qb, 4 * qb + 4):
                for nh in range(2):
                    steps.append(o_step(tb, nh))
            return steps

        def emit_attention_stream(queue, prologue):
            """All query blocks' attention chunks as ONE flat stream:
            (qb, p, kc) in causal order, PV lagging S/exp by TWO chunks
            so the next chunk's S (which gates the next exp) always runs
            during the current exp -- ScalarE never waits on queue pops
            or PV. One queue step pops per chunk, after the S. Phase
            boundaries get no pipeline flush; pv accumulators allocate
            lazily at each phase's first PV."""

            def emit_S_exp_mask(qb, p, kc):
                m = max(0, kc - 4 * qb)
                q0 = 128 * m
                ps_s = spsum.tile([128, 1024], FP32, name="ps_s",
                                  tag="ps_s", bufs=2)
                ksl = slice(128 * kc, 128 * (kc + 1))
                for hh in range(2):
                    h = 2 * p + hh
                    pb = 32 * h
                    for th in range(2):
                        if 256 * (th + 1) <= q0:
                            continue   # entire 256-col q-range masked;
                            # the exp below skips these cols too
                        nc.tensor.matmul(
                            ps_s[:, 512 * hh + 256 * th:
                                 512 * hh + 256 * (th + 1)],
                            lhsT=cur["kt8"][pb:pb + 32, :, ksl],
                            rhs=cur["qt8"][pb:pb + 32, :,
                                    512 * qb + 256 * th:
                                    512 * qb + 256 * (th + 1)],
                            start=True, stop=True,
                            perf_mode=DR,
                            tile_position=(pb, 0),
                        )
                pt = ptpool.tile([128, 1024], BF16, name="pt")
                if q0 < 256:
                    # single contiguous exp over [q0, 1024): covers head
                    # A's valid cols [q0,512) and head B's [512+q0,1024);
                    # cols [512,512+q0) hold exp(stale-but-written psum)
                    # and are never read
                    nc.scalar.activation(pt[:, q0:1024], ps_s[:, q0:1024],
                                         EXP, scale=EXP_SCALE)
                else:
                    # th=0 S instrs were skipped: exp only written ranges
                    for hh in range(2):
                        sl = slice(512 * hh + q0, 512 * (hh + 1))
                        nc.scalar.activation(pt[:, sl], ps_s[:, sl],
                                             EXP, scale=EXP_SCALE)
                diag = kc >= 4 * qb
                if diag:
                    # triangular mask only touches the 128-col diagonal
                    # block [q0, q0+128); later cols are fully valid
                    for i in range(2):
                        sl = slice(512 * i + q0, 512 * i + q0 + 128)
                        nc.gpsimd.affine_select(
                            out=pt[:, sl], in_=pt[:, sl],
                            compare_op=mybir.AluOpType.is_ge,
                            fill=0.0,
                            base=0,
                            pattern=[[1, 128]],
                            channel_multiplier=-1,
                        )
                return pt, q0, diag

            def emit_PV(qb, p, kc, pt, q0, diag, pv_a, pv_b):
                # exactly one start (first instr of kc==0) and one stop
                # (last instr of kc==nkc-1) per accumulator: psum "start"
                # marks the whole 2KB zero region pending-zero, so later
                # first-touches of other columns still get zeroed.
                st = (kc == 0)
                last = (kc == 4 * (qb + 1) - 1)
                for hh, pv in ((0, pv_a), (1, pv_b)):
                    base = 512 * hh
                    if diag:
                        if q0 + 128 < 512:   # clean part, no mask dep
                            nc.tensor.matmul(
                                pv[:, q0 + 128:512],
                                lhsT=cur["vt"][:, kc, 2 * p + hh, :],
                                rhs=pt[:, base + q0 + 128:base + 512],
                                start=st, stop=False,
                            )
                            st = False
                        # masked 128-col diagonal block
                        nc.tensor.matmul(
                            pv[:, q0:q0 + 128],
                            lhsT=cur["vt"][:, kc, 2 * p + hh, :],
                            rhs=pt[:, base + q0:base + q0 + 128],
                            start=st, stop=last,
                        )
                    else:
                        nc.tensor.matmul(
                            pv[:, 0:512],
                            lhsT=cur["vt"][:, kc, 2 * p + hh, :],
                            rhs=pt[:, base:base + 512],
                            start=st, stop=False,
                        )
                    st = (kc == 0)   # reset for the hh=1 accumulator

            def make_epilogue2(qb, p, st_a, st_b):
                qsl = slice(512 * qb, 512 * (qb + 1))

                def go():
                    rec = smpool.tile([HD + 1, 1024], F32R, name="rec")
                    with nc.allow_low_precision(reason="f32r softmax recip"):
                        nc.vector.reciprocal(rec[HD:HD + 1, 0:512],
                                             st_a[HD:HD + 1, :])
                        nc.vector.reciprocal(rec[HD:HD + 1, 512:1024],
                                             st_b[HD:HD + 1, :])
                    bc_a = mmpsum.tile([HD, 512], FP32, name="bc_a", tag="ps")
                    bc_b = mmpsum.tile([HD, 512], FP32, name="bc_b", tag="ps")
                    ones_ap = ones128[HD:HD + 1, :]
                    nc.tensor.matmul(bc_a[:, :], lhsT=ones_ap,
                                     rhs=rec[HD:HD + 1, 0:512],
                                     start=True, stop=True)
                    nc.tensor.matmul(bc_b[:, :], lhsT=ones_ap,
                                     rhs=rec[HD:HD + 1, 512:1024],
                                     start=True, stop=True)
                    # head 2p lands on ctxT partitions 0-63 directly
                    nc.vector.tensor_mul(ctxT[0:HD, p, qsl], st_a[0:HD, :],
                                         bc_a[:, :])
                    # head 2p+1: multiply at partitions 0-63, DMA to 64-127
                    cb = stpool.tile([HD, 512], BF16, name="cb", tag="cb")
                    nc.vector.tensor_mul(cb[:, :], st_b[0:HD, :], bc_b[:, :])
                    # gpsimd-issued DMA: keeps the latency-critical ctxT
                    # relocation off the busy sync queue
                    nc.gpsimd.dma_start(out=ctxT[HD:128, p, qsl],
                                        in_=cb[:, :])
                return go

            pv = {}        # (qb, p) -> (pv_a, pv_b), allocated lazily

            def flush_one():
                qb_, p_, kc_, pt_, q0_, diag_ = pending.pop(0)
                if kc_ == 0:
                    pv[(qb_, p_)] = (
                        pvpsum.tile([HD + 1, 512], FP32, name="pv_a",
                                    tag="pv_a"),
                        pvpsum.tile([HD + 1, 512], FP32, name="pv_b",
                                    tag="pv_b"),
                    )
                pv_a, pv_b = pv[(qb_, p_)]
                emit_PV(qb_, p_, kc_, pt_, q0_, diag_, pv_a, pv_b)
                if kc_ == 4 * (qb_ + 1) - 1:
                    # epilogue stage 1: stage PSUM out (frees pv slots);
                    # stage 2 (normalize into ctxT) goes to the queue head
                    st_a = stpool.tile([HD + 1, 512], FP32, name="st_a",
                                       tag="st")
                    st_b = stpool.tile([HD + 1, 512], FP32, name="st_b",
                                       tag="st")
                    nc.vector.tensor_copy(st_a[:, :], pv_a[:, :])
                    nc.vector.tensor_copy(st_b[:, :], pv_b[:, :])
                    queue.insert(0, make_epilogue2(qb_, p_, st_a, st_b))
                    del pv[(qb_, p_)]

            chunks = [(qb, p, kc) for qb in range(NQ) for p in range(2)
                      for kc in range(4 * (qb + 1))]
            pending = []
            last_qb = -1
            for qb, p, kc in chunks:
                if qb != last_qb:
                    for fn in prologue.get(qb, ()):
                        fn()
                    last_qb = qb
                pending.append((qb, p, kc) + emit_S_exp_mask(qb, p, kc))
                if len(pending) >= 3:
                    if queue:
                        queue.pop(0)()   # fill exp slack with queued work
                    flush_one()
            while pending:
                if queue:
                    queue.pop(0)()
                flush_one()

        def emit_body():
            queue = []
            # DMA priority: the first QK step needs wq8 + x8(0) only
            nc.sync.dma_start(out=wq8[:, :, :, :], in_=wq_d)
            emit_x8_dma(0, half=0)
            emit_x8_dma(0, half=1)
            nc.sync.dma_start(out=wk8[:, :, :, :], in_=wk_d)
            emit_xt_dma(0)
            emit_weights_vo()
            emit_x8_dma(1)
            emit_xt_dma(1)
            for s in make_qk_steps(0):
                s()
            for s in make_v_steps(0, tccs=[0]):
                s()
            # next block's Q proj runs serially up front: its DVE eviction
            # must land before attention(1)'s first S or the exp stream
            # stalls at the block boundary behind the DVE backlog
            for s in make_qk_steps(1, only="q"):
                s()
            # queue distribution balances each attention block's PE load
            # (S+PV+pops) against its exp budget (8*(qb+1) chunks): v(ib)
            # and outproj(qb) defer as late as deps allow -- PV touches
            # vt[kc] only ~kc chunks in, outproj(qb) only needs ctxT(qb).
            # v(ib) steps must stay at the queue head: v[tcc j] has to pop
            # before the PV that reads vt[j] (PE executes in order).
            queue += make_v_steps(0, tccs=[1, 2, 3]) + make_qk_steps(1, only="k")
            emit_x8_dma(2)
            emit_xt_dma(2)

            def pro1():
                emit_x8_dma(3)
                emit_xt_dma(3)
                queue.extend(make_qk_steps(2, only="q") + make_v_steps(1)
                             + make_qk_steps(2, only="k"))

            def pro2():
                queue.extend(make_qk_steps(3, only="q") + make_v_steps(2)
                             + make_qk_steps(3, only="k")
                             + make_outproj_steps(0))

            def pro3():
                queue.extend(make_v_steps(3) + make_outproj_steps(1)
                             + make_outproj_steps(2))

            emit_attention_stream(queue, {1: [pro1], 2: [pro2], 3: [pro3]})
            queue += make_outproj_steps(3)
            while queue:
                queue.pop(0)()

        for _rep in range(repeat):
            emit_body()

    nc.compile()
    return nc


def _get_nc(repeat=1):
    key = ("nc", repeat)
    if key not in _CACHE:
        _CACHE[key] = _build_nc(repeat)
    return _CACHE[key]


def _np_f8():
    import concourse.mybir as mybir
    return mybir.dt.np(mybir.dt.float8e4)


def _make_in_maps(x, Wq, Wk, Wv, Wo):
    import ml_dtypes
    bf = ml_dtypes.bfloat16
    f8 = _np_f8()
    x = np.asarray(x, dtype=np.float32)
    in_maps = []

    def dr_w(Wg):
        """[1024, 256] -> fp8 DoubleRow layout [128, 4, 2, 256] with
        columns permuted to (d_half, head, d_local%32)."""
        Wp = Wg.reshape(D, HG, 2, 32).transpose(0, 2, 1, 3).reshape(D, DG)
        return np.ascontiguousarray(
            Wp.reshape(NF // 2, 2, 128, DG).transpose(2, 0, 1, 3)).astype(f8)

    for c in range(NCORES):
        b, g = divmod(c, 4)
        cs = slice(DG * g, DG * (g + 1))
        xT = np.ascontiguousarray(x[b].T)                       # [1024, N]
        xt_bf = np.ascontiguousarray(
            xT.reshape(NF, 128, N).transpose(1, 0, 2)).astype(bf)
        x8 = np.ascontiguousarray(
            xT.reshape(NF // 2, 2, 128, N).transpose(2, 0, 1, 3)).astype(f8)
        wv = np.ascontiguousarray(
            np.asarray(Wv[:, cs], np.float32).reshape(NF, 128, DG)
            .transpose(1, 0, 2)).astype(bf)
        wo = np.ascontiguousarray(
            np.asarray(Wo[cs, :], np.float32).reshape(2, 128, D)
            .transpose(1, 0, 2)).astype(bf)
        in_maps.append({
            "x8": x8,
            "xt": xt_bf,
            "wq8": dr_w(32.0 * np.asarray(Wq[:, cs], np.float32)),
            "wk8": dr_w(32.0 * np.asarray(Wk[:, cs], np.float32)),
            "wv": wv,
            "wo": wo,
        })
    return in_maps


def _gather(results, bo):
    out = np.empty((2, N, D), dtype=np.float32)
    for b in range(2):
        acc = results[4 * b]["out"].astype(np.float32)
        for g in range(1, 4):
            acc = acc + results[4 * b + g]["out"].astype(np.float32)
        out[b] = acc + bo[None, :].astype(np.float32)
    return out


def run_spmd(x, Wq, Wk, Wv, Wo, bo, **spmd_kwargs):
    """Run the 8-core kernel; returns (full_output, BassKernelResults)."""
    from concourse.bass_utils import run_bass_kernel_spmd

    nc = _get_nc()
    in_maps = _make_in_maps(
        np.asarray(x), np.asarray(Wq), np.asarray(Wk), np.asarray(Wv),
        np.asarray(Wo))
    res = run_bass_kernel_spmd(nc, in_maps, core_ids=list(range(NCORES)),
                               **spmd_kwargs)
    return _gather(res.results, np.asarray(bo)), res


def kernel(x, Wq, Wk, Wv, Wo, bo):
    out, _ = run_spmd(x, Wq, Wk, Wv, Wo, bo)
    return out
